# revision 1
# baseline (speedup 1.0000x reference)
"""GNN message-passing (gated GCN style) on 8 Trainium2 NeuronCores.

Strategy (edge-parallel, dst-sorted shards):
- Host sorts edges by dst and splits into 8 shards snapped to node-run
  boundaries, so each device owns a contiguous node range and its complete
  incoming-edge runs. segment_max is fully local.
- Per device, nodes are sorted by in-degree and each node's run is padded to
  a per-tile power-of-2 slot count S, so segment_max becomes a fixed-window
  reduce_max over contiguous columns (feat-major).
- Per layer, each device computes [h@V | h@C | h@B] for its own node slice
  only; [h@V | h@C] is AllGathered so the per-edge src-side gather is one
  indirect DMA per 128 edges. h@B (dst side, sorted) is expanded with a
  constant kron-pattern matmul instead of a gather.
- BatchNorm statistics are masked sums reduced on-chip and combined with a
  single small AllReduce per layer.
- The readout MLP runs feat-major per 512-edge group; h@W0b / h@W0c are
  pre-folded into the final AllGather payload / local table.
"""

import numpy as np

NC = 8
D = 128


# ---------------------------------------------------------------------------
# host-side planning
# ---------------------------------------------------------------------------


def _next_pow2(x):
    p = 1
    while p < x:
        p *= 2
    return p


def _plan(src, dst, N):
    E = src.shape[0]
    order = np.argsort(dst, kind="stable")
    dsts = dst[order]
    srcs = src[order]

    # shard boundaries snapped to run starts
    bounds = [0]
    for r in range(1, NC):
        t = (E * r) // NC
        b = int(np.searchsorted(dsts, dsts[t], side="left"))
        bounds.append(max(b, bounds[-1]))
    bounds.append(E)

    lo = np.zeros(NC, np.int64)
    for d in range(1, NC):
        lo[d] = int(dsts[bounds[d]]) if bounds[d] < E else N
    hi = np.empty(NC, np.int64)
    hi[:-1] = lo[1:]
    hi[-1] = N

    n_r = [int(hi[d] - lo[d]) for d in range(NC)]
    NODE_CAP = 128 * int(np.ceil((max(n_r) + 2) / 128))
    T = NODE_CAP // 128

    lo = np.asarray(lo)
    shards = []
    for d in range(NC):
        sl = slice(bounds[d], bounds[d + 1])
        dl = dsts[sl] - lo[d]
        cnt = np.bincount(dl, minlength=n_r[d]) if n_r[d] > 0 else np.zeros(0, int)
        starts = np.concatenate([[0], np.cumsum(cnt)])
        perm = np.argsort(-cnt, kind="stable") if n_r[d] > 0 else np.zeros(0, int)
        ipos = np.empty(n_r[d], np.int64)
        ipos[perm] = np.arange(n_r[d])
        shards.append(
            dict(sl=sl, dl=dl, cnt=cnt, starts=starts, perm=perm, ipos=ipos, d=d)
        )

    # shared per-tile slot counts
    S_list = []
    for t in range(T):
        mx = 1
        for sh in shards:
            p = sh["perm"][t * 128 : (t + 1) * 128]
            if len(p):
                c = sh["cnt"][p]
                if len(c):
                    mx = max(mx, int(c.max()))
        S_list.append(min(_next_pow2(mx), 128))

    E_PAD = 128 * int(np.sum(S_list))
    C_E = E_PAD // 128
    chunks = []  # (tile, ci, S)
    for t in range(T):
        for ci in range(S_list[t]):
            chunks.append((t, ci, S_list[t]))
    groups = []  # (c0, gsz)
    c = 0
    while c < C_E:
        g = min(4, C_E - c)
        groups.append((c, g))
        c += g

    return dict(
        E=E,
        N=N,
        order=order,
        srcs=srcs,
        bounds=bounds,
        lo=np.array(lo),
        hi=np.array(hi),
        n_r=n_r,
        NODE_CAP=NODE_CAP,
        T=T,
        S_list=S_list,
        E_PAD=E_PAD,
        C_E=C_E,
        chunks=chunks,
        groups=groups,
        shards=shards,
    )


def _per_core_arrays(plan, d, h, e):
    """Build padded per-core host arrays for shard d."""
    sh = plan["shards"][d]
    NODE_CAP, T = plan["NODE_CAP"], plan["T"]
    S_list = plan["S_list"]
    E_PAD, C_E = plan["E_PAD"], plan["C_E"]
    lo = plan["lo"]
    n_r = plan["n_r"][d]
    guard_row = d * NODE_CAP + (NODE_CAP - 1)

    e_sh = e[plan["order"]][sh["sl"]]  # [E_s, F_E]
    src_sh = plan["srcs"][sh["sl"]]
    orig_sh = np.arange(plan["E"])[plan["order"]][sh["sl"]]

    F_E = e.shape[1]
    e0_pad = np.zeros((E_PAD, F_E), np.float32)
    srcrow = np.full(E_PAD, guard_row, np.int64)
    maskf = np.zeros(E_PAD, np.float32)
    origid = np.full(E_PAD, -1, np.int64)

    base = 0
    perm = sh["perm"]
    cnt = sh["cnt"]
    starts = sh["starts"]
    rank_of = lambda g: np.clip(
        np.searchsorted(plan["lo"], g, side="right") - 1, 0, NC - 1
    )
    for t in range(T):
        S = S_list[t]
        pn = perm[t * 128 : (t + 1) * 128]
        # index matrix [128, S] of local edge positions, -1 = dummy
        im = np.full((128, S), -1, np.int64)
        for i, n in enumerate(pn):
            dg = int(cnt[n])
            k = min(dg, S)
            if k:
                im[i, :k] = np.arange(starts[n], starts[n] + k)
        flat = im.reshape(-1)
        real = flat >= 0
        fr = flat[real]
        blk = slice(base, base + 128 * S)
        e0_blk = np.zeros((128 * S, F_E), np.float32)
        e0_blk[real] = e_sh[fr]
        e0_pad[blk] = e0_blk
        sr = np.full(128 * S, guard_row, np.int64)
        g = src_sh[fr]
        r = rank_of(g)
        loc = g - plan["lo"][r]
        pp_ = np.empty(len(g), np.int64)
        for rr in np.unique(r):
            m = r == rr
            pp_[m] = plan["shards"][rr]["ipos"][loc[m]]
        sr[real] = r * NODE_CAP + pp_
        srcrow[blk] = sr
        mk = np.zeros(128 * S, np.float32)
        mk[real] = 1.0
        maskf[blk] = mk
        oi = np.full(128 * S, -1, np.int64)
        oi[real] = orig_sh[fr]
        origid[blk] = oi
        base += 128 * S

    # srcidx [128, C_E]: edge (c*128+p) -> srcrow
    srcidx = srcrow.reshape(C_E, 128).T.astype(np.int32).copy()
    # invperm [128, T]: perm position (t*128+p) -> natural local row
    invp = np.full((T * 128,), NODE_CAP - 2, np.int64)
    invp[: len(perm)] = perm
    invperm = invp.reshape(T, 128).T.astype(np.int32).copy()
    # mask_e [NG, 512]
    NG = len(plan["groups"])
    maske = np.zeros((NG, 512), np.float32)
    for gi, (c0, gsz) in enumerate(plan["groups"]):
        maske[gi, : gsz * 128] = maskf[c0 * 128 : (c0 + gsz) * 128]
    # nodemask [T, 128]: perm position real?
    nm = np.zeros(T * 128, np.float32)
    nm[: n_r] = 1.0
    nodemask = nm.reshape(T, 128).astype(np.float32)
    # h0T [F_N, NODE_CAP] permuted
    F_N = h.shape[1]
    h0p = np.zeros((NODE_CAP, F_N), np.float32)
    hl = h[plan["lo"][d] : plan["hi"][d]]
    h0p[: len(perm)] = hl[perm]
    h0T = np.ascontiguousarray(h0p.T)

    return dict(
        h0T=h0T,
        e0T=np.ascontiguousarray(e0_pad.T),
        srcidx=srcidx,
        invperm=invperm,
        maske=maske,
        nodemask=nodemask,
        origid=origid,
    )


# ---------------------------------------------------------------------------
# device program
# ---------------------------------------------------------------------------


def _build_program(plan, stop_after="full", ablate=None):
    import concourse.bass as bass
    import concourse.mybir as mybir
    import concourse.tile as tile
    from concourse import bacc
    from concourse.bass import IndirectOffsetOnAxis

    F32 = mybir.dt.float32
    I32 = mybir.dt.int32
    AF = mybir.ActivationFunctionType
    OP = mybir.AluOpType
    AX = mybir.AxisListType

    NODE_CAP, T = plan["NODE_CAP"], plan["T"]
    E_PAD, C_E = plan["E_PAD"], plan["C_E"]
    chunks, groups = plan["chunks"], plan["groups"]
    NG = len(groups)
    N, E = plan["N"], plan["E"]
    S_vals = sorted(set(plan["S_list"]))
    kron_of = {s: i for i, s in enumerate(S_vals)}
    F_N, F_E = plan["F_N"], plan["F_E"]
    EPS = 1e-5

    _phases = ["embed", "bound0", "epass0", "layer0", "layer1", "full"]
    if stop_after == "gath0":
        lvl = 1
    elif stop_after in ("va0", "vb0"):
        lvl = 2
    else:
        lvl = _phases.index(stop_after)
    agg_on = stop_after != "vb0"
    zpath_on = stop_after != "va0"

    nc = bacc.Bacc(
        "TRN2", target_bir_lowering=False, debug=False, num_devices=NC
    )

    def din(name, shape, dt=F32):
        return nc.dram_tensor(name, shape, dt, kind="ExternalInput")

    # per-core inputs
    h0T = din("h0T", [F_N, NODE_CAP])
    e0T = din("e0T", [F_E, E_PAD])
    srcidx = din("srcidx", [128, C_E], I32)
    invperm = din("invperm", [128, T], I32)
    maske = din("maske", [NG, 512])
    nodemask = din("nodemask", [T, 128])
    # shared weights
    ident_d = din("ident", [128, 128])
    ones_d = din("ones_row", [1, 128])
    gneg_d = din("gneg", [1, 256])
    emb_e_w = din("emb_e_w", [F_E, 128])
    emb_n_w = din("emb_n_w", [F_N, 128])
    emb_e_b = din("emb_e_b", [128, 1])
    emb_n_b = din("emb_n_b", [128, 1])
    A_d = [din(f"A{l}", [128, 128]) for l in range(2)]
    VCB0_d = din("VCB0", [128, 384])
    V1_d = din("V1", [128, 128])
    U_d = [din(f"U{l}", [128, 128]) for l in range(2)]
    WBC_d = din("WBC", [128, 256])
    W0a_d = din("W0a", [128, 128])
    W0b_col = din("W0b_col", [128, 1])
    Wk_d = [din(f"Wk{k}", [128, 128]) for k in range(2)]
    Wkb_col = [din(f"Wkb{k}", [128, 1]) for k in range(2)]
    Wf_d = din("Wf", [128, 1])
    wfb_d = din("wfb", [1, 1])
    krons_d = din("krons", [len(S_vals), 128, 128])

    y_out = nc.dram_tensor("y", [1, E_PAD], F32, kind="ExternalOutput")

    rg = [list(range(NC))]

    with tile.TileContext(nc) as tc:
        with (
            tc.tile_pool(name="const", bufs=1) as cp,
            tc.tile_pool(name="pers", bufs=1) as pp,
            tc.tile_pool(name="st", bufs=1) as stp,
            tc.tile_pool(name="s", bufs=2) as sp,
            tc.tile_pool(name="ps", bufs=2, space="PSUM") as ps,
            tc.tile_pool(name="dram", bufs=1, space="DRAM") as dp,
        ):
            # ---- load constants
            def cload(dram_t, shape, dt=F32, name=None):
                t = cp.tile(shape, dt, name=name or dram_t.name + "_sb")
                nc.sync.dma_start(out=t[:], in_=dram_t[:])
                return t

            ident = cload(ident_d, [128, 128])
            ones_row = cload(ones_d, [1, 128])
            embe_w = cload(emb_e_w, [F_E, 128])
            embn_w = cload(emb_n_w, [F_N, 128])
            embe_b = cload(emb_e_b, [128, 1])
            embn_b = cload(emb_n_b, [128, 1])
            A_sb = [cload(A_d[l], [128, 128]) for l in range(2)]
            VCB0 = cload(VCB0_d, [128, 384])
            V1 = cload(V1_d, [128, 128])
            U_sb = [cload(U_d[l], [128, 128]) for l in range(2)]
            WBC = cload(WBC_d, [128, 256])
            W0a = cload(W0a_d, [128, 128])
            W0bc = cload(W0b_col, [128, 1])
            Wk = [cload(Wk_d[k], [128, 128]) for k in range(2)]
            Wkb = [cload(Wkb_col[k], [128, 1]) for k in range(2)]
            Wf = cload(Wf_d, [128, 1])
            wfb = cload(wfb_d, [1, 1])
            kron_sb = []
            for i in range(len(S_vals)):
                kt = cp.tile([128, 128], F32, name=f"kron{i}")
                nc.sync.dma_start(out=kt[:], in_=krons_d[i])
                kron_sb.append(kt)
            srci = cload(srcidx, [128, C_E], I32)
            invp = cload(invperm, [128, T], I32)
            eps_col = cp.tile([128, 1], F32, name="eps_col")
            nc.gpsimd.memset(eps_col[:], EPS)

            # ---- dram buffers
            e_buf = [
                dp.tile([128, E_PAD], F32, name=f"e_buf{i}") for i in range(2)
            ]
            z_buf = dp.tile([128, E_PAD], F32, name="z_buf")
            hb_buf = dp.tile([NODE_CAP, 128], F32, name="hb_buf")
            hfm_buf = [
                dp.tile([128, NODE_CAP], F32, name=f"hfm_buf{i}")
                for i in range(3)
            ]
            hlocal = dp.tile([NODE_CAP, 128], F32, name="hlocal")
            cc_hin = [
                dp.tile([NODE_CAP, 256 if l == 0 else 128], F32, name=f"cc_hin{l}")
                for l in range(3)
            ]
            cc_hout = [
                dp.tile(
                    [NC * NODE_CAP, 256 if l == 0 else 128],
                    F32,
                    name=f"cc_hout{l}",
                    addr_space="Shared",
                )
                for l in range(3)
            ]
            cc_st_in = [
                dp.tile([128, 4 if l == 0 else 2], F32, name=f"cc_st_in{l}")
                for l in range(2)
            ]
            cc_st_out = [
                dp.tile(
                    [128, 4 if l == 0 else 2],
                    F32,
                    name=f"cc_st_out{l}",
                    addr_space="Shared",
                )
                for l in range(2)
            ]
            cc_moy_in = dp.tile([128, 1], F32, name="cc_moy_in")
            cc_moy_out = dp.tile([128, 1], F32, name="cc_moy_out", addr_space="Shared")

            # ---- persistent sbuf tiles
            hU = [None] * T
            agg = [None] * T

            # ================= embed e =================
            for gi, (c0, gsz) in enumerate(groups):
                w = gsz * 128
                e0sl = sp.tile([F_E, 512], F32, tag="e0sl")
                nc.sync.dma_start(
                    out=e0sl[:, :w], in_=e0T[:, c0 * 128 : c0 * 128 + w]
                )
                pe = ps.tile([128, 512], F32, tag="pa")
                nc.tensor.matmul(
                    out=pe[:, :w], lhsT=embe_w[:], rhs=e0sl[:, :w],
                    start=True, stop=True,
                )
                esb = sp.tile([128, 512], F32, tag="esb", bufs=4)
                nc.scalar.activation(
                    out=esb[:, :w], in_=pe[:, :w], func=AF.Identity,
                    bias=embe_b[:],
                )
                nc.sync.dma_start(
                    out=e_buf[0][:, c0 * 128 : c0 * 128 + w], in_=esb[:, :w]
                )

            # ================= embed h =================
            for t in range(T):
                h0sl = sp.tile([F_N, 128], F32, tag="h0sl", bufs=2, name="h0sl")
                nc.sync.dma_start(
                    out=h0sl[:], in_=h0T[:, t * 128 : (t + 1) * 128]
                )
                ph = ps.tile([128, 128], F32, tag="pc")
                nc.tensor.matmul(
                    out=ph[:], lhsT=embn_w[:], rhs=h0sl[:],
                    start=True, stop=True,
                )
                hf = sp.tile([128, 128], F32, tag="hnew", bufs=4, name="hemb")
                nc.scalar.activation(
                    out=hf[:], in_=ph[:], func=AF.Identity, bias=embn_b[:]
                )
                nc.sync.dma_start(
                    out=hfm_buf[0][:, t * 128 : (t + 1) * 128], in_=hf[:]
                )

            # ================= boundary =================
            def boundary(l):
                """Build tables for layer l (or readout if l==2) from hfm."""
                if l == 0:
                    rhs, wdt, U = VCB0, 384, U_sb[0]
                elif l == 1:
                    rhs, wdt, U = V1, 128, U_sb[1]
                else:
                    rhs, wdt, U = WBC, 256, None
                for t in range(T):
                    hfl = sp.tile([128, 128], F32, tag="hfl", bufs=4, name="hfl")
                    nc.sync.dma_start(
                        out=hfl[:], in_=hfm_buf[l][:, t * 128 : (t + 1) * 128]
                    )
                    pb = ps.tile([128, 512], F32, tag="pa")
                    nc.tensor.matmul(
                        out=pb[:, :wdt], lhsT=hfl[:], rhs=rhs[:],
                        start=True, stop=True,
                    )
                    bsb = sp.tile([128, 512], F32, tag="bsb", bufs=4)
                    nc.scalar.activation(
                        out=bsb[:, :wdt], in_=pb[:, :wdt], func=AF.Copy
                    )
                    scat_w = 256 if l == 0 else 128
                    nc.sync.dma_start(
                        out=cc_hin[l][t * 128 : (t + 1) * 128, :scat_w],
                        in_=bsb[:, :scat_w],
                    )
                    if l == 0:
                        nc.sync.dma_start(
                            out=hb_buf[t * 128 : (t + 1) * 128, :],
                            in_=bsb[:, 256:384],
                        )
                    if l == 2:
                        nc.sync.dma_start(
                            out=hlocal[t * 128 : (t + 1) * 128, :],
                            in_=bsb[:, 128:256],
                        )
                    if U is not None:
                        pu = ps.tile([128, 128], F32, tag="pc")
                        nc.tensor.matmul(
                            out=pu[:], lhsT=U[:], rhs=hfl[:],
                            start=True, stop=True,
                        )
                        hu = pp.tile(
                            [128, 128], F32, tag=f"hU{t}", name=f"hU_{t}"
                        )
                        nc.scalar.activation(out=hu[:], in_=pu[:], func=AF.Copy)
                        hU[t] = hu
                # guard row
                nc.sync.dma_start(
                    out=cc_hin[l][NODE_CAP - 1 : NODE_CAP, :],
                    in_=gneg_d[:, : (256 if l == 0 else 128)],
                )
                nc.gpsimd.collective_compute(
                    "AllGather",
                    OP.bypass,
                    replica_groups=rg,
                    ins=[cc_hin[l][:]],
                    outs=[cc_hout[l][:]],
                )

            if lvl >= 1:
                boundary(0)

            # ================= layers =================
            stat_cols = None
            for l in range(2):
                if l == 0 and lvl < 2:
                    break
                if l == 1 and lvl < 4:
                    break
                gwidth = 256 if l == 0 else 128
                # stats accumulators
                if l == 0:
                    ssum_e = stp.tile([128, NG], F32, name="ssum_e")
                    ssq_e = stp.tile([128, NG], F32, name="ssq_e")
                hsum = stp.tile([128, T], F32, name=f"hsum{l}")
                hssq = stp.tile([128, T], F32, name=f"hssq{l}")

                # ---- e-pass (layer 1's is fused into the l==0 e-update)
                for gi, (c0, gsz) in enumerate(groups if l == 0 else []):
                    w = gsz * 128
                    esb = sp.tile([128, 512], F32, tag="esb", bufs=4)
                    nc.sync.dma_start(
                        out=esb[:, :w],
                        in_=e_buf[l][:, c0 * 128 : c0 * 128 + w],
                    )
                    wsb = sp.tile([128, 512], F32, tag="wsb", bufs=3)
                    nc.scalar.activation(
                        out=wsb[:, :w], in_=esb[:, :w], func=AF.Sigmoid
                    )
                    if l == 0 and zpath_on:
                        mrow = sp.tile([1, 512], F32, tag="mrow", bufs=2)
                        nc.sync.dma_start(out=mrow[:], in_=maske[gi : gi + 1, :])
                        pm = ps.tile([128, 512], F32, tag="pb")
                        nc.tensor.matmul(
                            out=pm[:, :w], lhsT=ones_row[:], rhs=mrow[:, :w],
                            start=True, stop=True,
                        )
                        pz = ps.tile([128, 512], F32, tag="pa")
                        pd = ps.tile([128, 512], F32, tag="pdy", bufs=2, name="pd")
                        nc.tensor.matmul(
                            out=pz[:, :w], lhsT=A_sb[l][:], rhs=esb[:, :w],
                            start=True, stop=False, skip_group_check=True,
                        )
                    for k in range(gsz):
                        c = c0 + k
                        t, ci, S = chunks[c]
                        G = 128 // S
                        gt = sp.tile([128, 256], F32, tag="gt", bufs=16)
                        if ablate == 'nogather':
                            nc.vector.memset(gt[:, :128], 0.5)
                        else:
                            nc.gpsimd.indirect_dma_start(
                                out=gt[:, :gwidth],
                                out_offset=None,
                                in_=cc_hout[l][:],
                                in_offset=IndirectOffsetOnAxis(
                                    ap=srci[:, c : c + 1], axis=0
                                ),
                            )
                        if agg_on:
                            phv = ps.tile([128, 128], F32, tag="pc")
                            nc.tensor.matmul(
                                out=phv[:], lhsT=gt[:, :128], rhs=ident[:],
                                is_transpose=True, start=True, stop=True,
                                skip_group_check=True,
                            )
                            msg = sp.tile([128, 128], F32, tag="msg", bufs=6)
                            nc.vector.tensor_tensor(
                                out=msg[:], in0=phv[:],
                                in1=wsb[:, k * 128 : (k + 1) * 128], op=OP.mult,
                            )
                            if ci == 0:
                                ag = pp.tile(
                                    [128, 128], F32, tag=f"agg{t}", name=f"agg_{t}"
                                )
                                agg[t] = ag
                            nc.vector.tensor_reduce(
                                out=agg[t][:, ci * G : (ci + 1) * G],
                                in_=msg[:].rearrange("p (g s) -> p g s", s=S),
                                op=OP.max,
                                axis=AX.X,
                            )
                        if l == 0 and zpath_on:
                            nc.tensor.matmul(
                                out=pd[:, k * 128 : (k + 1) * 128],
                                lhsT=gt[:, 128:256],
                                rhs=ident[:],
                                is_transpose=True,
                                start=True, stop=True, skip_group_check=True,
                            )
                            band = sp.tile([128, 128], F32, tag="hbt", bufs=4, name="band")
                            nc.sync.dma_start(
                                out=band[:G, :],
                                in_=hb_buf[
                                    t * 128 + ci * G : t * 128 + ci * G + G, :
                                ],
                            )
                            nc.tensor.matmul(
                                out=pz[:, k * 128 : (k + 1) * 128],
                                lhsT=band[:G, :],
                                rhs=kron_sb[kron_of[S]][:G, :],
                                start=False, stop=(k == gsz - 1),
                                skip_group_check=True,
                            )
                    if l == 0 and zpath_on:
                        zraw = sp.tile([128, 512], F32, tag="zraw", bufs=3)
                        nc.scalar.activation(
                            out=zraw[:, :w], in_=pz[:, :w], func=AF.Copy
                        )
                        zsum = sp.tile([128, 512], F32, tag="zsum", bufs=3)
                        nc.vector.tensor_tensor(
                            out=zsum[:, :w], in0=zraw[:, :w], in1=pd[:, :w],
                            op=OP.add,
                        )
                        zm = sp.tile([128, 512], F32, tag="zm", bufs=3)
                        nc.vector.tensor_tensor(
                            out=zm[:, :w], in0=zsum[:, :w], in1=pm[:, :w],
                            op=OP.mult,
                        )
                        nc.vector.tensor_reduce(
                            out=ssum_e[:, gi : gi + 1], in_=zm[:, :w],
                            op=OP.add, axis=AX.X,
                        )
                        sq = sp.tile([128, 512], F32, tag="sq", bufs=3)
                        nc.scalar.activation(
                            out=sq[:, :w], in_=zm[:, :w], func=AF.Square
                        )
                        nc.vector.tensor_reduce(
                            out=ssq_e[:, gi : gi + 1], in_=sq[:, :w],
                            op=OP.add, axis=AX.X,
                        )
                        nc.sync.dma_start(
                            out=z_buf[:, c0 * 128 : c0 * 128 + w],
                            in_=zm[:, :w],
                        )

                if l == 0 and lvl < 3:
                    break

                # ---- h side: z_h = hU + select(agg); masked stats
                for t in range(T):
                    m01 = sp.tile([128, 128], F32, tag="m01", bufs=2)
                    nc.vector.tensor_scalar(
                        out=m01[:], in0=agg[t][:], scalar1=-1e20, scalar2=None,
                        op0=OP.is_gt,
                    )
                    nc.vector.tensor_tensor(
                        out=agg[t][:], in0=agg[t][:], in1=m01[:], op=OP.mult
                    )
                    nc.vector.tensor_tensor(
                        out=agg[t][:], in0=agg[t][:], in1=hU[t][:], op=OP.add
                    )
                    nmr = sp.tile([1, 128], F32, tag="nmr", bufs=2)
                    nc.sync.dma_start(out=nmr[:], in_=nodemask[t : t + 1, :])
                    pnm = ps.tile([128, 128], F32, tag="pc")
                    nc.tensor.matmul(
                        out=pnm[:], lhsT=ones_row[:], rhs=nmr[:],
                        start=True, stop=True, skip_group_check=True,
                    )
                    zhm = sp.tile([128, 128], F32, tag="zhm", bufs=2)
                    nc.vector.tensor_tensor(
                        out=zhm[:], in0=agg[t][:], in1=pnm[:], op=OP.mult
                    )
                    nc.vector.tensor_reduce(
                        out=hsum[:, t : t + 1], in_=zhm[:], op=OP.add, axis=AX.X
                    )
                    sqh = sp.tile([128, 128], F32, tag="sqh", bufs=2)
                    nc.scalar.activation(
                        out=sqh[:], in_=zhm[:], func=AF.Square
                    )
                    nc.vector.tensor_reduce(
                        out=hssq[:, t : t + 1], in_=sqh[:], op=OP.add,
                        axis=AX.X,
                    )

                # ---- pack + allreduce stats
                ncols = 4 if l == 0 else 2
                pack = stp.tile([128, 4], F32, name=f"pack{l}")
                nc.vector.tensor_reduce(
                    out=pack[:, 0:1], in_=hsum[:], op=OP.add, axis=AX.X
                )
                nc.vector.tensor_reduce(
                    out=pack[:, 1:2], in_=hssq[:], op=OP.add, axis=AX.X
                )
                if l == 0:
                    nc.vector.tensor_reduce(
                        out=pack[:, 2:3], in_=ssum_e[:], op=OP.add, axis=AX.X
                    )
                    nc.vector.tensor_reduce(
                        out=pack[:, 3:4], in_=ssq_e[:], op=OP.add, axis=AX.X
                    )
                nc.sync.dma_start(out=cc_st_in[l][:], in_=pack[:, :ncols])
                nc.gpsimd.collective_compute(
                    "AllReduce", OP.add, replica_groups=rg,
                    ins=[cc_st_in[l][:]], outs=[cc_st_out[l][:]],
                )
                stt = stp.tile([128, 4], F32, name=f"stt{l}")
                nc.sync.dma_start(out=stt[:, :ncols], in_=cc_st_out[l][:])

                # ---- bn coefficients
                def bn_cols(sum_c, ssq_c, count, pref):
                    mean = stp.tile([128, 1], F32, name=f"{pref}mean{l}")
                    nc.vector.tensor_scalar(
                        out=mean[:], in0=sum_c, scalar1=1.0 / count,
                        scalar2=None, op0=OP.mult,
                    )
                    msq = stp.tile([128, 1], F32, name=f"{pref}msq{l}")
                    nc.vector.tensor_scalar(
                        out=msq[:], in0=ssq_c, scalar1=1.0 / count,
                        scalar2=None, op0=OP.mult,
                    )
                    m2 = stp.tile([128, 1], F32, name=f"{pref}m2{l}")
                    nc.scalar.activation(out=m2[:], in_=mean[:], func=AF.Square)
                    var = stp.tile([128, 1], F32, name=f"{pref}var{l}")
                    nc.vector.tensor_tensor(
                        out=var[:], in0=msq[:], in1=m2[:], op=OP.subtract
                    )
                    sd = stp.tile([128, 1], F32, name=f"{pref}sd{l}")
                    nc.scalar.activation(
                        out=sd[:], in_=var[:], func=AF.Sqrt, bias=eps_col[:]
                    )
                    rs = stp.tile([128, 1], F32, name=f"{pref}rs{l}")
                    nc.vector.reciprocal(out=rs[:], in_=sd[:])
                    bb = stp.tile([128, 1], F32, name=f"{pref}bb{l}")
                    nc.vector.tensor_tensor(
                        out=bb[:], in0=mean[:], in1=rs[:], op=OP.mult
                    )
                    nc.vector.tensor_scalar(
                        out=bb[:], in0=bb[:], scalar1=-1.0, scalar2=None,
                        op0=OP.mult,
                    )
                    return rs, bb

                rs_h, bb_h = bn_cols(stt[:, 0:1], stt[:, 1:2], N, "h")
                if l == 0:
                    rs_e, bb_e = bn_cols(stt[:, 2:3], stt[:, 3:4], E, "e")

                # ---- h update
                for t in range(T):
                    r = sp.tile([128, 128], F32, tag="rh", bufs=4)
                    nc.scalar.activation(
                        out=r[:], in_=agg[t][:], func=AF.Relu,
                        bias=bb_h[:], scale=rs_h[:],
                    )
                    hfl = sp.tile([128, 128], F32, tag="hfl", bufs=4, name="hflu")
                    nc.sync.dma_start(
                        out=hfl[:], in_=hfm_buf[l][:, t * 128 : (t + 1) * 128]
                    )
                    hf2 = sp.tile([128, 128], F32, tag="hnew", bufs=4, name="hupd")
                    nc.vector.tensor_tensor(
                        out=hf2[:], in0=hfl[:], in1=r[:], op=OP.add
                    )
                    nc.sync.dma_start(
                        out=hfm_buf[l + 1][:, t * 128 : (t + 1) * 128],
                        in_=hf2[:],
                    )

                boundary(l + 1)

                # ---- e update (only needed after layer 0)
                if l == 0:
                    for gi, (c0, gsz) in enumerate(groups):
                        w = gsz * 128
                        zsb = sp.tile([128, 512], F32, tag="zsb", bufs=4)
                        nc.sync.dma_start(
                            out=zsb[:, :w],
                            in_=z_buf[:, c0 * 128 : c0 * 128 + w],
                        )
                        r = sp.tile([128, 512], F32, tag="re", bufs=3)
                        nc.scalar.activation(
                            out=r[:, :w], in_=zsb[:, :w], func=AF.Relu,
                            bias=bb_e[:], scale=rs_e[:],
                        )
                        eold = sp.tile([128, 512], F32, tag="esb", bufs=4)
                        nc.sync.dma_start(
                            out=eold[:, :w],
                            in_=e_buf[0][:, c0 * 128 : c0 * 128 + w],
                        )
                        enew = sp.tile([128, 512], F32, tag="enew", bufs=3)
                        nc.vector.tensor_tensor(
                            out=enew[:, :w], in0=eold[:, :w], in1=r[:, :w],
                            op=OP.add,
                        )
                        # fused layer-1 message pass: w1 = sigmoid(e1) and
                        # the hV1[src] gather + segment-max, no e round trip
                        w1 = sp.tile([128, 512], F32, tag="wsb", bufs=3)
                        nc.scalar.activation(
                            out=w1[:, :w], in_=enew[:, :w], func=AF.Sigmoid
                        )
                        for k in range(gsz):
                            c = c0 + k
                            t, ci, S = chunks[c]
                            G = 128 // S
                            gt = sp.tile([128, 256], F32, tag="gt", bufs=16)
                            if ablate == 'nogather':
                                nc.vector.memset(gt[:, :128], 0.5)
                            else:
                                nc.gpsimd.indirect_dma_start(
                                    out=gt[:, :128],
                                    out_offset=None,
                                    in_=cc_hout[1][:],
                                    in_offset=IndirectOffsetOnAxis(
                                        ap=srci[:, c : c + 1], axis=0
                                    ),
                                )
                            phv = ps.tile([128, 128], F32, tag="pc")
                            nc.tensor.matmul(
                                out=phv[:], lhsT=gt[:, :128], rhs=ident[:],
                                is_transpose=True, start=True, stop=True,
                                skip_group_check=True,
                            )
                            msg = sp.tile([128, 128], F32, tag="msg", bufs=6)
                            nc.vector.tensor_tensor(
                                out=msg[:], in0=phv[:],
                                in1=w1[:, k * 128 : (k + 1) * 128], op=OP.mult,
                            )
                            if ci == 0:
                                ag = pp.tile(
                                    [128, 128], F32, tag=f"agg{t}",
                                    name=f"agg1_{t}",
                                )
                                agg[t] = ag
                            nc.vector.tensor_reduce(
                                out=agg[t][:, ci * G : (ci + 1) * G],
                                in_=msg[:].rearrange("p (g s) -> p g s", s=S),
                                op=OP.max,
                                axis=AX.X,
                            )

            # ================= moy + base =================
            if lvl < 5:
                ydummy = sp.tile([1, E_PAD], F32, tag="ydummy", bufs=1)
                nc.gpsimd.memset(ydummy[:], 0.0)
                nc.sync.dma_start(out=y_out[:], in_=ydummy[:])
                if stop_after == "gath0":
                    gtd = sp.tile([128, 256], F32, tag="gt", bufs=2)
                    nc.gpsimd.indirect_dma_start(
                        out=gtd[:],
                        out_offset=None,
                        in_=cc_hout[0][:],
                        in_offset=IndirectOffsetOnAxis(
                            ap=srci[:, 0:1], axis=0
                        ),
                    )
                    nc.sync.dma_start(
                        out=y_out[0:1, 0:256], in_=gtd[0:1, :]
                    )
            else:
                moysum = stp.tile([128, T], F32, name="moysum")
                for t in range(T):
                    nmr = sp.tile([1, 128], F32, tag="nmr", bufs=2)
                    nc.sync.dma_start(out=nmr[:], in_=nodemask[t : t + 1, :])
                    pnm = ps.tile([128, 128], F32, tag="pc")
                    nc.tensor.matmul(
                        out=pnm[:], lhsT=ones_row[:], rhs=nmr[:],
                        start=True, stop=True, skip_group_check=True,
                    )
                    hfl = sp.tile([128, 128], F32, tag="hfl", bufs=4, name="hflm")
                    nc.sync.dma_start(
                        out=hfl[:], in_=hfm_buf[2][:, t * 128 : (t + 1) * 128]
                    )
                    hm = sp.tile([128, 128], F32, tag="zhm", bufs=2)
                    nc.vector.tensor_tensor(
                        out=hm[:], in0=hfl[:], in1=pnm[:], op=OP.mult
                    )
                    nc.vector.tensor_reduce(
                        out=moysum[:, t : t + 1], in_=hm[:], op=OP.add, axis=AX.X
                    )
                moyp = stp.tile([128, 1], F32, name="moyp")
                nc.vector.tensor_reduce(
                    out=moyp[:], in_=moysum[:], op=OP.add, axis=AX.X
                )
                nc.sync.dma_start(out=cc_moy_in[:], in_=moyp[:])
                nc.gpsimd.collective_compute(
                    "AllReduce", OP.add, replica_groups=rg,
                    ins=[cc_moy_in[:]], outs=[cc_moy_out[:]],
                )
                moyc = stp.tile([128, 1], F32, name="moyc")
                nc.sync.dma_start(out=moyc[:], in_=cc_moy_out[:])
                nc.vector.tensor_scalar(
                    out=moyc[:], in0=moyc[:], scalar1=1.0 / N, scalar2=None,
                    op0=OP.mult,
                )
                pbase = ps.tile([128, 128], F32, tag="pc")
                nc.tensor.matmul(
                    out=pbase[:, 0:1], lhsT=W0a[:], rhs=moyc[:],
                    start=True, stop=True, skip_group_check=True,
                )
                base_col = stp.tile([128, 1], F32, name="base_col")
                nc.vector.tensor_tensor(
                    out=base_col[:], in0=pbase[:, 0:1], in1=W0bc[:], op=OP.add
                )

                # ================= readout =================
                for gi, (c0, gsz) in enumerate(groups):
                    w = gsz * 128
                    pt1 = ps.tile([128, 512], F32, tag="pa")
                    for k in range(gsz):
                        c = c0 + k
                        t, ci, S = chunks[c]
                        G = 128 // S
                        gt = sp.tile([128, 256], F32, tag="gt", bufs=16)
                        if ablate == 'nogather':
                            nc.vector.memset(gt[:, :128], 0.5)
                        else:
                            nc.gpsimd.indirect_dma_start(
                                out=gt[:, :128],
                                out_offset=None,
                                in_=cc_hout[2][:],
                                in_offset=IndirectOffsetOnAxis(
                                    ap=srci[:, c : c + 1], axis=0
                                ),
                            )
                        nc.tensor.matmul(
                            out=pt1[:, k * 128 : (k + 1) * 128], lhsT=gt[:, :128],
                            rhs=ident[:], is_transpose=True,
                            start=True, stop=False, skip_group_check=True,
                        )
                        band = sp.tile([128, 128], F32, tag="hbt", bufs=4, name="band")
                        nc.sync.dma_start(
                            out=band[:G, :],
                            in_=hlocal[t * 128 + ci * G : t * 128 + ci * G + G, :],
                        )
                        nc.tensor.matmul(
                            out=pt1[:, k * 128 : (k + 1) * 128],
                            lhsT=band[:G, :],
                            rhs=kron_sb[kron_of[S]][:G, :],
                            start=False, stop=True, skip_group_check=True,
                        )
                    t1 = sp.tile([128, 512], F32, tag="t1", bufs=3)
                    nc.scalar.activation(
                        out=t1[:, :w], in_=pt1[:, :w], func=AF.Relu,
                        bias=base_col[:],
                    )
                    pt2 = ps.tile([128, 512], F32, tag="pb")
                    nc.tensor.matmul(
                        out=pt2[:, :w], lhsT=Wk[0][:], rhs=t1[:, :w],
                        start=True, stop=True, skip_group_check=True,
                    )
                    t2 = sp.tile([128, 512], F32, tag="t2", bufs=3)
                    nc.scalar.activation(
                        out=t2[:, :w], in_=pt2[:, :w], func=AF.Relu, bias=Wkb[0][:]
                    )
                    pt3 = ps.tile([128, 512], F32, tag="pa")
                    nc.tensor.matmul(
                        out=pt3[:, :w], lhsT=Wk[1][:], rhs=t2[:, :w],
                        start=True, stop=True, skip_group_check=True,
                    )
                    t3 = sp.tile([128, 512], F32, tag="t3", bufs=3)
                    nc.scalar.activation(
                        out=t3[:, :w], in_=pt3[:, :w], func=AF.Relu, bias=Wkb[1][:]
                    )
                    py = ps.tile([1, 512], F32, tag="pdy", bufs=2, name="py")
                    nc.tensor.matmul(
                        out=py[:, :w], lhsT=Wf[:], rhs=t3[:, :w],
                        start=True, stop=True, skip_group_check=True,
                    )
                    ysb = sp.tile([1, 512], F32, tag="ysb", bufs=2)
                    nc.scalar.activation(
                        out=ysb[:, :w], in_=py[:, :w], func=AF.Sigmoid,
                        bias=wfb[:],
                    )
                    nc.sync.dma_start(
                        out=y_out[0:1, c0 * 128 : c0 * 128 + w], in_=ysb[:, :w]
                    )

    nc.compile()
    return nc


# ---------------------------------------------------------------------------
# top level
# ---------------------------------------------------------------------------


def _make_kron(S):
    G = 128 // S
    k = np.zeros((128, 128), np.float32)
    for p in range(128):
        g = p % G
        k[p, g * S : (g + 1) * S] = 1.0
    return k


def kernel(**inputs):
    import sys

    if "/opt/trn_rl_repo" not in sys.path:
        sys.path.insert(0, "/opt/trn_rl_repo")
    from concourse.bass_utils import run_bass_kernel_spmd

    h = np.asarray(inputs["h"], np.float32)
    e = np.asarray(inputs["e"], np.float32)
    src = np.asarray(inputs["src"]).astype(np.int64)
    dst = np.asarray(inputs["dst"]).astype(np.int64)
    N = h.shape[0]
    E = e.shape[0]

    plan = _plan(src, dst, N)
    plan["F_N"] = h.shape[1]
    plan["F_E"] = e.shape[1]

    U = np.asarray(inputs["U"], np.float32)
    V = np.asarray(inputs["V"], np.float32)
    A = np.asarray(inputs["A"], np.float32)
    B = np.asarray(inputs["B"], np.float32)
    C = np.asarray(inputs["C"], np.float32)
    W0_w = np.asarray(inputs["W0_w"], np.float32)
    Wk_w = np.asarray(inputs["Wk_w"], np.float32)
    Wk_b = np.asarray(inputs["Wk_b"], np.float32)
    Wf_w = np.asarray(inputs["Wf_w"], np.float32)
    Wf_b = np.asarray(inputs["Wf_b"], np.float32)

    S_vals = sorted(set(plan["S_list"]))
    krons = np.stack([_make_kron(s) for s in S_vals])

    shared = dict(
        ident=np.eye(128, dtype=np.float32),
        ones_row=np.ones((1, 128), np.float32),
        gneg=np.full((1, 256), -1e30, np.float32),
        emb_e_w=np.asarray(inputs["emb_e_w"], np.float32),
        emb_n_w=np.asarray(inputs["emb_n_w"], np.float32),
        emb_e_b=np.asarray(inputs["emb_e_b"], np.float32).reshape(128, 1),
        emb_n_b=np.asarray(inputs["emb_n_b"], np.float32).reshape(128, 1),
        A0=np.ascontiguousarray(A[0]),
        A1=np.ascontiguousarray(A[1]),
        VCB0=np.ascontiguousarray(
            np.concatenate([V[0], C[0], B[0]], axis=1)
        ),
        V1=np.ascontiguousarray(V[1]),
        U0=np.ascontiguousarray(U[0]),
        U1=np.ascontiguousarray(U[1]),
        WBC=np.ascontiguousarray(
            np.concatenate([W0_w[128:256], W0_w[256:384]], axis=1)
        ),
        W0a=np.ascontiguousarray(W0_w[:128]),
        W0b_col=np.asarray(inputs["W0_b"], np.float32).reshape(128, 1),
        Wk0=np.ascontiguousarray(Wk_w[0]),
        Wk1=np.ascontiguousarray(Wk_w[1]),
        Wkb0=Wk_b[0].reshape(128, 1).astype(np.float32),
        Wkb1=Wk_b[1].reshape(128, 1).astype(np.float32),
        Wf=Wf_w.reshape(128, 1).astype(np.float32),
        wfb=np.full((1, 1), float(Wf_b), np.float32),
        krons=krons,
    )

    in_maps = []
    origids = []
    for d in range(NC):
        pc = _per_core_arrays(plan, d, h, e)
        origids.append(pc.pop("origid"))
        m = dict(pc)
        m.update(shared)
        in_maps.append(m)

    nc = _build_program(plan)
    res = run_bass_kernel_spmd(nc, in_maps, list(range(NC)))

    out = np.zeros(E, np.float32)
    for d in range(NC):
        y = np.asarray(res.results[d]["y"]).reshape(-1)
        oid = origids[d]
        valid = oid >= 0
        out[oid[valid]] = y[valid]
    return out

